# revision 1
# baseline (speedup 1.0000x reference)
"""Trainium2 Bass kernel for nn_Dependence_Learning (sparse_attention).

Computation (faithful to the reference's row-major reshapes):
  tri1 = relu(BN(x @ w1))   # key map
  tri2 = relu(BN(x @ w2))   # query map
  Flat t-major space per (b, ch): F[t*4096 + l] = shifted image (9 shifts of
  the k=3 window, zero padded).  The reference's scrambled reshapes make the
  softmax run over consecutive GROUPS OF 9 of this flat array, with the
  "center" at group offset 4:
    lg[p] = F1[p]*F2[9*(p//9)+4] + F1[9*(p//9)+4]*F2[p]
    SM    = softmax over each group of 9
    pre[l] = sum_t SM[t*4096+l] * Fx[t*4096+l]
  out = relu(BN(pre @ wf))

Sharding: each of the 8 cores owns 32 of the 256 channels for conv1/conv2 and
the attention stage (BN stats are per-channel -> fully local), then an
AllGather of `pre` lets each core compute 32 output channels of the final
conv (again with local BN stats).

Per-core pair layout: partition p = 32*b + j holds (batch b, local channel j).

The attention stage runs over 4 column quarters x 9 shift segments.  For
quarter q / segment t the native flat range [4096 t + 1024 q, +1024) is
extended both ways to group-of-9 boundaries; every group is complete inside
the extent and only the native range is accumulated.  Each quarter's `pre`
columns are AllGathered as soon as they finish, overlapping the remaining
attention compute, and the final conv consumes the gathered quarters as they
arrive.

Matmuls run in float32r (~1.6e-4 rel err, 4x fp32 throughput).
"""

import numpy as np
from contextlib import ExitStack

import concourse.bass as bass
import concourse.bacc as bacc
import concourse.tile as tile
import concourse.mybir as mybir
from concourse.bass_utils import run_bass_kernel_spmd

F32 = mybir.dt.float32
F32R = mybir.dt.float32r
AF = mybir.ActivationFunctionType
AX = mybir.AxisListType

NCORES = 8
B = 4
C = 256
HH = 64
L = HH * HH           # 4096
CS = C // NCORES      # 32 channels per core
P = 128               # partitions; == B * CS
PH = HH + 2           # 66 padded
PSZ = PH * PH         # 4356
NQ = 4                # column quarters
QW = L // NQ          # 1024
QR = QW // HH         # 16 rows per quarter
# uneven column chunks: big ones amortize, small ones shrink the exit tail
CHUNKS = [(0, 1024), (1024, 1024), (2048, 1024), (3072, 768), (3840, 256)]
NCH = len(CHUNKS)
NLC = 8               # conv l chunks
LCW = L // NLC        # 512
NSTAT = B * L         # 16384 samples per channel for BN
EPS = 1e-5
EXPB = -30.0          # constant exp bias (softmax shift; ratios unchanged)
EXTMAX = max(w for _, w in CHUNKS) + 16      # >= 1024 + r + s

assert L % 9 == 1


def _shift(t):
    return t // 3 - 1, t % 3 - 1


def build_program():
    nc = bacc.Bacc("TRN2", target_bir_lowering=False, num_devices=NCORES)

    x_t = nc.dram_tensor("x", [B, C, L], F32R, kind="ExternalInput")
    xs_t = nc.dram_tensor("xs", [P, L], F32, kind="ExternalInput")
    w1t_t = nc.dram_tensor("w1t", [C, CS], F32R, kind="ExternalInput")
    w2t_t = nc.dram_tensor("w2t", [C, CS], F32R, kind="ExternalInput")
    wfl_t = nc.dram_tensor("wfl", [2, CS, P], F32R, kind="ExternalInput")
    bnp_t = nc.dram_tensor("bnp", [6, CS], F32, kind="ExternalInput")
    out_t = nc.dram_tensor("out", [P, L], F32, kind="ExternalOutput")

    with tile.TileContext(nc) as tc, ExitStack() as top:
        consts = top.enter_context(tc.tile_pool(name="consts", bufs=1))
        persist = top.enter_context(tc.tile_pool(name="persist", bufs=1))
        tiny = top.enter_context(tc.tile_pool(name="tiny", bufs=4))
        dram = top.enter_context(tc.tile_pool(name="dram", bufs=1, space="DRAM"))

        # ---- weights / constants ----
        # conv weights, zero-padded so batch b's 32 out channels sit at
        # column band 32b (f32r matmul cannot column-tile the dst)
        wt = {}
        for name, t in (("w1", w1t_t), ("w2", w2t_t)):
            for h in range(2):
                for b in range(B):
                    w = consts.tile([P, P], F32R, tag=f"{name}p{h}{b}",
                                    name=f"{name}p{h}{b}")
                    nc.vector.memset(w[:, :].bitcast(F32), 0.0)
                    nc.sync.dma_start(out=w[:, CS * b:CS * (b + 1)],
                                      in_=t[h * P:(h + 1) * P, :])
                    wt[(name, h, b)] = w
        # wf for the partial final conv: [32, 128] block replicated on all
        # four 32-partition bands (lhsT rows must sit on the rhs partitions)
        wfl = []
        for h in range(2):
            wr = consts.tile([P, P], F32R, tag=f"wflh{h}", name=f"wflh{h}")
            nc.gpsimd.dma_start(
                out=wr[:, :],
                in_=bass.AP(tensor=wfl_t, offset=h * CS * P,
                            ap=[[0, B], [P, CS], [1, P]]))
            wfl.append(wr)
        eps_c = consts.tile([P, 1], F32, tag="eps_c")
        nc.vector.memset(eps_c[:, :], EPS)
        expb_c = consts.tile([P, 1], F32, tag="expb_c")
        nc.vector.memset(expb_c[:, :], EXPB)
        # selector for cross-b stat combine: sel[p, j] = 1 if p % 32 == j
        sel_t = nc.inline_tensor(
            np.tile(np.eye(CS, dtype=np.float32), (B, 1)), name="sel")
        sel = consts.tile([P, CS], F32, tag="sel")
        nc.gpsimd.dma_start(out=sel[:, :], in_=sel_t[:, :])
        id_t = nc.inline_tensor(np.eye(P, dtype=np.float32), name="idm")
        idm = consts.tile([P, P], F32R, tag="idm")
        nc.gpsimd.dma_start(out=idm[:, :], in_=id_t[:, :].bitcast(F32R))
        selT_t = nc.inline_tensor(
            np.tile(np.eye(CS, dtype=np.float32), (1, B)), name="selT")
        selT = consts.tile([CS, P], F32, tag="selT")
        nc.gpsimd.dma_start(out=selT[:, :], in_=selT_t[:, :])
        # BN params as per-partition columns [32, 1] (g_i, be_i for i=0,1,2)
        bncol = []
        for i in range(6):
            col = consts.tile([CS, 1], F32, tag=f"bncol{i}", name=f"bncol{i}")
            nc.sync.dma_start(
                out=col[:, :],
                in_=bass.AP(tensor=bnp_t, offset=i * CS, ap=[[1, CS], [0, 1]]))
            bncol.append(col)

        ab = [persist.tile([P, 2], F32, tag=f"ab{i}", name=f"ab{i}")
              for i in range(3)]
        sums = [persist.tile([P, NLC], F32, tag=f"sums{i}", name=f"sums{i}")
                for i in range(4)]
        sumsf = [persist.tile([P, NCH], F32, tag=f"sumsf{i}", name=f"sumsf{i}")
                 for i in range(2)]

        partd = [dram.tile([NCORES * P, w], F32, tag=f"partd{ci}",
                           name=f"partd{ci}")
                 for ci, (c0, w) in enumerate(CHUNKS)]
        recvd = [dram.tile([P, w], F32, tag=f"recvd{ci}", name=f"recvd{ci}")
                 for ci, (c0, w) in enumerate(CHUNKS)]

        def bn_coeffs(sum_ap, sumsq_ap, pidx, ab_out, n):
            """Per-channel [32,1] stats -> ab_out[:, 0:1] = g/sqrt(var+eps),
            ab_out[:, 1:2] = be - mean*a.  pidx picks the (g, be) pair."""
            mean = tiny.tile([CS, 1], F32, tag="mean")
            var = tiny.tile([CS, 1], F32, tag="var")
            sd = tiny.tile([CS, 1], F32, tag="sd")
            tmp = tiny.tile([CS, 1], F32, tag="tmp")
            nc.vector.tensor_scalar_mul(mean[:, :], sum_ap, 1.0 / n)
            nc.vector.tensor_scalar_mul(var[:, :], sumsq_ap, 1.0 / n)
            nc.vector.tensor_mul(tmp[:, :], mean[:, :], mean[:, :])
            nc.vector.tensor_sub(var[:, :], var[:, :], tmp[:, :])
            nc.scalar.activation(sd[:, :], var[:, :], AF.Sqrt,
                                 bias=eps_c[0:CS, :])
            nc.vector.reciprocal(sd[:, :], sd[:, :])
            nc.vector.tensor_mul(ab_out[:, 0:1], bncol[2 * pidx][:, :], sd[:, :])
            nc.vector.tensor_mul(tmp[:, :], mean[:, :], ab_out[:, 0:1])
            nc.vector.tensor_sub(ab_out[:, 1:2], bncol[2 * pidx + 1][:, :],
                                 tmp[:, :])

        with tc.tile_pool(name="imgs", bufs=1) as imgs:
            img1 = imgs.tile([P, PSZ], F32, tag="img1")
            img2 = imgs.tile([P, PSZ], F32, tag="img2")
            img3 = imgs.tile([P, PSZ], F32, tag="img3")
            i3d = {}
            for nm, im in (("1", img1), ("2", img2), ("3", img3)):
                v = im[:, :].rearrange("p (r c) -> p r c", c=PH)
                i3d[nm] = v
                nc.vector.memset(v[:, 0, :], 0.0)
                nc.vector.memset(v[:, PH - 1, :], 0.0)
                nc.vector.memset(v[:, 1:PH - 1, 0], 0.0)
                nc.vector.memset(v[:, 1:PH - 1, PH - 1], 0.0)

            # x pair-layout load into img3 interior -- emitted on the gpsimd
            # (SWDGE) queue so it never delays the conv rhs stream on sync
            nc.gpsimd.dma_start(
                out=i3d["3"][:, 1:1 + HH, 1:1 + HH],
                in_=bass.AP(tensor=xs_t, offset=0,
                            ap=[[L, P], [HH, HH], [1, HH]]),
            )

            # ================= phase 1: conv1 + conv2 + BN stats =========
            with ExitStack() as s1:
                rhsp = s1.enter_context(tc.tile_pool(name="rhs", bufs=6))
                psump = s1.enter_context(
                    tc.tile_pool(name="psum", bufs=2, space="PSUM"))
                evict = s1.enter_context(tc.tile_pool(name="evict", bufs=2))
                ybuf = s1.enter_context(tc.tile_pool(name="ybuf", bufs=1))
                y1s = ybuf.tile([P, L], F32, tag="y1")
                y2s = ybuf.tile([P, L], F32, tag="y2")

                for lc in range(NLC):
                    p1 = psump.tile([P, LCW], F32, tag="p1")
                    p2 = psump.tile([P, LCW], F32, tag="p2")
                    for b in range(B):
                        rt = []
                        for h in range(2):
                            r = rhsp.tile([P, LCW], F32R, tag="rhs")
                            nc.sync.dma_start(
                                out=r[:, :],
                                in_=bass.AP(
                                    tensor=x_t,
                                    offset=b * C * L + h * P * L + lc * LCW,
                                    ap=[[L, P], [1, LCW]]),
                            )
                            rt.append(r)
                        for h in range(2):
                            nc.tensor.matmul(
                                p1[:, :], wt[("w1", h, b)][:, :],
                                rt[h][:, :],
                                start=(b == 0 and h == 0),
                                stop=(b == B - 1 and h == 1),
                                tile_position=(0, 0))
                        for h in range(2):
                            nc.tensor.matmul(
                                p2[:, :], wt[("w2", h, b)][:, :],
                                rt[h][:, :],
                                start=(b == 0 and h == 0),
                                stop=(b == B - 1 and h == 1),
                                tile_position=(0, 0))
                    for (pp, ys, si) in ((p1, y1s, 0), (p2, y2s, 2)):
                        nc.scalar.activation(
                            out=ys[:, lc * LCW:(lc + 1) * LCW], in_=pp[:, :],
                            func=AF.Copy, accum_out=sums[si][:, lc:lc + 1])
                        sq = evict.tile([P, LCW], F32, tag="sq")
                        nc.scalar.activation(
                            out=sq[:, :], in_=pp[:, :],
                            func=AF.Square, accum_out=sums[si + 1][:, lc:lc + 1])

                # ---- stats: fold chunks, combine across b via PE selector --
                stats4 = persist.tile([P, 4], F32, tag="stats4")
                for i in range(4):
                    nc.vector.reduce_sum(out=stats4[:, i:i + 1],
                                         in_=sums[i][:, :], axis=AX.X)
                stp = psump.tile([CS, 4], F32, tag="stp")
                nc.tensor.matmul(stp[:, :], sel[:, :], stats4[:, :],
                                 start=True, stop=True, tile_position=(0, 0))
                statjs = persist.tile([CS, 4], F32, tag="statjs")
                nc.scalar.activation(out=statjs[:, :], in_=stp[:, :],
                                     func=AF.Copy)
                abj = persist.tile([CS, 4], F32, tag="abj")
                for i in range(2):
                    bn_coeffs(statjs[:, 2 * i:2 * i + 1],
                              statjs[:, 2 * i + 1:2 * i + 2], i,
                              abj[:, 2 * i:2 * i + 2], NSTAT)
                ab_d = dram.tile([CS, 4], F32, tag="ab_d")
                nc.sync.dma_start(out=ab_d[:, :], in_=abj[:, :])
                for i in range(2):
                    nc.sync.dma_start(
                        out=ab[i][:, :],
                        in_=bass.AP(tensor=ab_d.tensor,
                                    offset=ab_d.offset + 2 * i,
                                    ap=[[0, B], [4, CS], [1, 2]]))

                # ---- BN apply + relu into padded interiors ----
                for (ys, im, i) in ((y1s, i3d["1"], 0), (y2s, i3d["2"], 1)):
                    nc.scalar.activation(
                        out=im[:, 1:1 + HH, 1:1 + HH],
                        in_=ys[:, :].rearrange("p (r c) -> p r c", c=HH),
                        func=AF.Relu,
                        bias=ab[i][:, 1:2], scale=ab[i][:, 0:1])

            # ===== phase 2+3: attention quarters, gathers, final conv =====
            with ExitStack() as s2:
                fpool = s2.enter_context(tc.tile_pool(name="fbuf", bufs=2))
                tpool = s2.enter_context(tc.tile_pool(name="tbuf", bufs=2))
                gpool = s2.enter_context(tc.tile_pool(name="gbuf", bufs=2))
                orpool = s2.enter_context(tc.tile_pool(name="outr", bufs=2))
                psumf = s2.enter_context(
                    tc.tile_pool(name="psumf", bufs=2, space="PSUM"))
                evictf = s2.enter_context(tc.tile_pool(name="evictf", bufs=2))
                ybuff = s2.enter_context(tc.tile_pool(name="ybuff", bufs=1))
                yfs = ybuff.tile([P, L], F32, tag="yf")
                fout = ybuff.tile([P, L], F32, tag="fout")

                OUTr_of = {}

                def sub_widths(w):
                    return [(i * LCW, min(LCW, w - i * LCW))
                            for i in range((w + LCW - 1) // LCW)]

                def partial_piece(ci, b, h, off, ww):
                    c0, w = CHUNKS[ci]
                    OUTr = OUTr_of[ci]
                    pf = psumf.tile([P, LCW], F32, tag="pf", name="pf")
                    nc.tensor.matmul(
                        pf[:, :ww],
                        wfl[h][CS * b:CS * (b + 1), :],
                        OUTr[CS * b:CS * (b + 1), off:off + ww],
                        start=True, stop=True,
                        tile_position=(CS * b, 0))
                    ev = evictf.tile([P, LCW], F32, tag="ev", name="ev")
                    nc.scalar.activation(out=ev[:, :ww], in_=pf[:, :ww],
                                         func=AF.Copy)
                    # partd addr(k=4h+a, p'=32b+j, l'); psum partition 32a+j
                    nc.sync.dma_start(
                        out=bass.AP(
                            tensor=partd[ci].tensor,
                            offset=(partd[ci].offset + h * 4 * P * w
                                    + b * CS * w + off),
                            ap=[[P * w, 4], [w, CS], [1, ww]]),
                        in_=ev[:, :ww])

                def emit_rs_cc(ci):
                    nc.gpsimd.collective_compute(
                        "ReduceScatter", mybir.AluOpType.add,
                        replica_groups=[list(range(NCORES))],
                        ins=[partd[ci][:, :].opt()],
                        outs=[recvd[ci][:, :].opt()])

                def emit_rs_post(ci):
                    c0, w = CHUNKS[ci]
                    nc.sync.dma_start(out=yfs[:, c0:c0 + w],
                                      in_=recvd[ci][:, :])
                    sq = evictf.tile([P, QW], F32, tag="sqf", name="sqf")
                    nc.scalar.activation(
                        out=sq[:, :w], in_=yfs[:, c0:c0 + w],
                        func=AF.Copy, accum_out=sumsf[0][:, ci:ci + 1])
                    nc.scalar.activation(
                        out=sq[:, :w], in_=yfs[:, c0:c0 + w],
                        func=AF.Square, accum_out=sumsf[1][:, ci:ci + 1])

                def pieces_of(ci):
                    return [(b, h, off, ww) for h in range(2)
                            for (off, ww) in sub_widths(CHUNKS[ci][1])
                            for b in range(B)]

                def emit_A(ci, t):
                    """one (chunk, shift-segment) iteration of the attention stage."""
                    c0, w = CHUNKS[ci]
                    row0, nrows = c0 // HH, w // HH
                    di, dj = _shift(t)
                    r = (t + c0) % 9
                    s = (-(t + c0 + w)) % 9
                    ext = w + r + s
                    ng = ext // 9
                    F1c = fpool.tile([P, EXTMAX], F32, tag="F1c", name="F1c")
                    F2c = fpool.tile([P, EXTMAX], F32, tag="F2c", name="F2c")
                    for (Fc, im, iflat) in ((F1c, i3d["1"], img1),
                                            (F2c, i3d["2"], img2)):
                        nc.scalar.copy(
                            out=Fc[:, r:r + w].rearrange(
                                "p (i j) -> p i j", j=HH),
                            in_=im[:, 1 + di + row0:1 + di + row0 + nrows,
                                   1 + dj:1 + dj + HH])
                        if r:
                            if c0:
                                off = (di + row0) * PH + (65 - r + dj)
                            else:
                                pdi, pdj = _shift(t - 1)
                                off = (64 + pdi) * PH + (65 - r + pdj)
                            nc.sync.dma_start(out=Fc[:, 0:r],
                                              in_=iflat[:, off:off + r])
                        if s:
                            if c0 + w < L:
                                off = (1 + di + row0 + nrows) * PH + 1 + dj
                            else:
                                ndi, ndj = _shift(t + 1)
                                off = (1 + ndi) * PH + (1 + ndj)
                            nc.sync.dma_start(out=Fc[:, r + w:r + w + s],
                                              in_=iflat[:, off:off + s])

                    F1g = F1c[:, :9 * ng].rearrange("p (g s) -> p g s", s=9)
                    F2g = F2c[:, :9 * ng].rearrange("p (g s) -> p g s", s=9)
                    # centers read straight from the F tiles
                    cqb = F2g[:, :, 4].unsqueeze(2).broadcast_to((P, ng, 9))
                    ckb = F1g[:, :, 4].unsqueeze(2).broadcast_to((P, ng, 9))

                    t1 = tpool.tile([P, EXTMAX], F32, tag="t1", name="t1")
                    t2 = tpool.tile([P, EXTMAX], F32, tag="t2", name="t2")
                    lgb = tpool.tile([P, EXTMAX], F32, tag="lgb", name="lgb")
                    ext = w + r + s
                    t1g = t1[:, :9 * ng].rearrange("p (g s) -> p g s", s=9)
                    t2g = t2[:, :9 * ng].rearrange("p (g s) -> p g s", s=9)
                    nc.vector.tensor_mul(t1g, F1g, cqb)
                    nc.vector.tensor_mul(t2g, F2g, ckb)
                    nc.vector.tensor_add(lgb[:, :ext], t1[:, :ext],
                                         t2[:, :ext])
                    nc.scalar.activation(out=t1[:, :ext], in_=lgb[:, :ext],
                                         func=AF.Exp, bias=expb_c[:, :])
                    S = gpool.tile([P, 116], F32, tag="S", name="S")
                    R = gpool.tile([P, 116], F32, tag="R", name="R")
                    nc.vector.reduce_sum(out=S[:, :ng], in_=t1g, axis=AX.X)
                    nc.vector.reciprocal(R[:, :ng], S[:, :ng])
                    nc.vector.tensor_mul(
                        t2g, t1g,
                        R[:, :ng].unsqueeze(2).broadcast_to((P, ng, 9)))
                    Pt = tpool.tile([P, QW], F32R, tag="Pt", name="Pt")
                    nc.vector.tensor_mul(
                        Pt[:, :w].rearrange("p (i j) -> p i j", j=HH),
                        t2[:, r:r + w].rearrange("p (i j) -> p i j", j=HH),
                        i3d["3"][:, 1 + di + row0:1 + di + row0 + nrows,
                                 1 + dj:1 + dj + HH])
                    OUTP = OUTq_cur[0]
                    for (off, ww) in sub_widths(w):
                        nc.tensor.matmul(
                            OUTP[:, off:off + ww],
                            idm[:, :],
                            Pt[:, off:off + ww],
                            start=(t == 0), stop=(t == 8),
                            tile_position=(0, 0), skip_group_check=True)

                OUTq_cur = [None]
                for ci, (c0, w) in enumerate(CHUNKS):
                    prev = pieces_of(ci - 1) if ci > 0 else []
                    OUTP = psumf.tile([P, QW], F32, tag="OUTP",
                                      name=f"OUTP{ci}")
                    OUTq_cur[0] = OUTP
                    for t in range(9):
                        # interleave previous chunk's partial final conv
                        if ci > 0 and t < 8:
                            for pc in prev[2 * t:2 * t + 2]:
                                partial_piece(ci - 1, *pc)
                        elif ci > 0 and t == 8:
                            emit_rs_cc(ci - 1)
                        if ci > 1 and t == 2:
                            emit_rs_post(ci - 2)
                        emit_A(ci, t)

                    # round pre chunk to f32r for the partial conv PE pass
                    OUTr = orpool.tile([P, QW], F32R, tag="OUTr",
                                       name="OUTr")
                    nc.vector.tensor_copy(out=OUTr[:, :w], in_=OUTP[:, :w])
                    OUTr_of[ci] = OUTr

                # drain the last chunk's partial conv + RS
                for pc in pieces_of(NCH - 1):
                    partial_piece(NCH - 1, *pc)
                emit_rs_cc(NCH - 1)
                emit_rs_post(NCH - 2)
                emit_rs_post(NCH - 1)

                # ---- final BN stats + apply + output ----
                stats2 = persist.tile([P, 2], F32, tag="stats2")
                for i in range(2):
                    nc.vector.reduce_sum(out=stats2[:, i:i + 1],
                                         in_=sumsf[i][:, :], axis=AX.X)
                stpf = psumf.tile([CS, 2], F32, tag="stpf", bufs=1)
                nc.tensor.matmul(stpf[:, :], sel[:, :], stats2[:, :],
                                 start=True, stop=True, tile_position=(0, 0))
                statjsf = persist.tile([CS, 2], F32, tag="statjsf")
                nc.scalar.activation(out=statjsf[:, :], in_=stpf[:, :],
                                     func=AF.Copy)
                abjf = persist.tile([CS, 2], F32, tag="abjf")
                bn_coeffs(statjsf[:, 0:1], statjsf[:, 1:2], 2,
                          abjf[:, 0:2], NSTAT)
                abpf = psumf.tile([P, 2], F32, tag="abpf", bufs=1)
                nc.tensor.matmul(abpf[:, :], selT[:, :], abjf[:, :],
                                 start=True, stop=True, tile_position=(0, 0))
                nc.scalar.activation(out=ab[2][:, :], in_=abpf[:, :],
                                     func=AF.Copy)
                nc.scalar.activation(out=fout[:, :], in_=yfs[:, :],
                                     func=AF.Relu,
                                     bias=ab[2][:, 1:2], scale=ab[2][:, 0:1])
                nc.sync.dma_start(out=out_t[:, :], in_=fout[:, :])

    nc.finalize()
    return nc


_NC_CACHE = None


def _get_nc():
    global _NC_CACHE
    if _NC_CACHE is None:
        _NC_CACHE = build_program()
    return _NC_CACHE


def make_in_maps(inputs):
    x = np.ascontiguousarray(np.asarray(inputs["x"], np.float32).reshape(B, C, L))
    maps = []
    for k in range(NCORES):
        sl = slice(k * CS, (k + 1) * CS)
        m = {
            "x": x,
            "xs": np.ascontiguousarray(x[:, sl, :].reshape(P, L)),
            "w1t": np.ascontiguousarray(np.asarray(inputs["w1"], np.float32)[sl, :].T),
            "w2t": np.ascontiguousarray(np.asarray(inputs["w2"], np.float32)[sl, :].T),
            "wfl": np.ascontiguousarray(np.stack([
                np.asarray(inputs["wf"], np.float32)[h * P:(h + 1) * P, sl].T
                for h in range(2)])),
            "bnp": np.ascontiguousarray(np.stack([
                np.asarray(inputs["g1"], np.float32)[sl],
                np.asarray(inputs["be1"], np.float32)[sl],
                np.asarray(inputs["g2"], np.float32)[sl],
                np.asarray(inputs["be2"], np.float32)[sl],
                np.asarray(inputs["gf"], np.float32)[sl],
                np.asarray(inputs["bef"], np.float32)[sl],
            ])),
        }
        maps.append(m)
    return maps


def run(inputs, trace=False):
    nc = _get_nc()
    in_maps = make_in_maps(inputs)
    res = run_bass_kernel_spmd(nc, in_maps, core_ids=list(range(NCORES)),
                               trace=trace)
    full = np.empty((B, C, HH, HH), np.float32)
    for k in range(NCORES):
        full[:, k * CS:(k + 1) * CS] = res.results[k]["out"].reshape(B, CS, HH, HH)
    return full, res


def kernel(**inputs) -> np.ndarray:
    out, _ = run(inputs, trace=False)
    return out



# revision 3
# speedup vs baseline: 1.7906x; 1.7906x over previous
"""Trainium2 Bass kernel for nn_Dependence_Learning (sparse_attention) — v2.

L-sharded design: each of 8 cores owns 8 image rows (512 pixels) of every
(batch, channel) pair.  Channels live on partitions (2 banks of 128).

  conv1/conv2: lhsT = w.T [in_ch part, out_ch], rhs = x [in_ch part, cols]
  BN stats: per-channel = per-partition bn_stats/bn_aggr over the owned
  region, then ONE AllGather of (mean, var) x 4 slots and a local combine.
  tri maps stored fp16 in a 65-col padded row layout (shared pad column
  gives correct zero for +-1 column shifts).

  Attention: the reference's scrambled reshape makes softmax run over
  groups of 9 consecutive positions of the flat space p = 4096 t + l.
  Per core the group phase r = (t - k) mod 9.  The loop runs over the
  PHASE rho (static geometry on every core); the shift used is
  t = (rho + k) mod 9, supplied per-core as a register offset table
  ("offt") that biases the tri / x source APs.  Cross-shift seam values
  (first/last <=8 positions, cores 0/7) come from a host-prepared patch
  strip ("xp") run through the same conv+BN path ("hp"), patched into the
  F tiles with one strided copy.

  Engine split per (rho, ob): Act: F overcopies + exp; Pool(gpsimd):
  the two center-broadcast muls; DVE: add, group-sum, recip, normalize,
  x-mul, patches; PE: identity-matmul accumulation of pre over the 9
  shifts into PSUM.

  Final conv like conv1 but from the accumulated pre (f32r), second
  AllGather for BNf stats, relu, DMA out.
"""

import numpy as np
from contextlib import ExitStack

import concourse.bass as bass
import concourse.bacc as bacc
import concourse.tile as tile
import concourse.mybir as mybir
from concourse.bass_utils import run_bass_kernel_spmd

F32 = mybir.dt.float32
F32R = mybir.dt.float32r
F16 = mybir.dt.float16
I32 = mybir.dt.int32
AF = mybir.ActivationFunctionType
AX = mybir.AxisListType

NCORES = 8
B = 4
C = 256
HH = 64
L = HH * HH            # 4096
P = 128
NB = 2                 # channel banks
RO = 8                 # owned rows per core
LC = RO * HH           # 512 owned pixels
RH = 12                # slab rows (halo 2 each side)
RX = 10                # x_att rows (halo 1 each side)
CC = B * RH * HH       # 3072 conv cols per bank
TRW = 1 + RH * 65      # 781   tri row-padded width per b
XAW = 1 + RX * 65      # 651
EPS = 1e-5
EXPB = -30.0
NPC = B * 9 * 16       # 576 patch-strip cols


def _shift(t):
    return t // 3 - 1, t % 3 - 1


def _geom(rho):
    s = (1 - rho) % 9
    ng = (rho + 512 + s) // 9
    return s, ng


def build_program():
    nc = bacc.Bacc("TRN2", target_bir_lowering=False, num_devices=NCORES)

    xs_t = nc.dram_tensor("xs", [NB * P, CC], F32R, kind="ExternalInput")
    xp_t = nc.dram_tensor("xp", [NB * P, NPC], F32R, kind="ExternalInput")
    wts_t = nc.dram_tensor("wts", [NB * P, 3 * C], F32R, kind="ExternalInput")
    bnq_t = nc.dram_tensor("bnq", [NB * P, 6], F32, kind="ExternalInput")
    hm_t = nc.dram_tensor("hmask", [P, 2], F32, kind="ExternalInput")
    hpm_t = nc.dram_tensor("hpmv", [P, 144], F16, kind="ExternalInput")
    offt_t = nc.dram_tensor("offt", [1, 18], I32, kind="ExternalInput")
    out_t = nc.dram_tensor("out", [NB * P, B * LC], F32, kind="ExternalOutput")

    with tile.TileContext(nc) as tc, ExitStack() as top:
        consts = top.enter_context(tc.tile_pool(name="consts", bufs=1))
        persist = top.enter_context(tc.tile_pool(name="persist", bufs=1))
        tiny = top.enter_context(tc.tile_pool(name="tiny", bufs=4))
        dram = top.enter_context(tc.tile_pool(name="dram", bufs=1, space="DRAM"))

        # ---- weights / constants ----
        # all conv weights in two [128, 768] tiles (one DMA each):
        # cols = (w1|w2|wf) * 256 + ob * 128 + co
        wbig = []
        for ib in range(NB):
            wb = consts.tile([P, 3 * C], F32R, tag=f"wbig{ib}",
                             name=f"wbig{ib}")
            nc.scalar.dma_start(out=wb[:, :],
                                in_=wts_t[ib * P:(ib + 1) * P, :])
            wbig.append(wb)
        _wi = {"w1": 0, "w2": 1, "wf": 2}

        def wt(key):
            name, ib, ob = key
            j = _wi[name] * C + ob * P
            return wbig[ib][:, j:j + P]
        # fp16 wf blocks for the fused final-conv accumulation
        wf16 = {}
        for ib in range(NB):
            for oo in range(NB):
                w16 = consts.tile([P, P], F16, tag=f"wf16_{ib}{oo}",
                                  name=f"wf16_{ib}{oo}")
                nc.vector.tensor_copy(out=w16[:, :],
                                      in_=wt(("wf", ib, oo)).bitcast(F32))
                wf16[(ib, oo)] = w16
        bnc = []
        for ob in range(NB):
            c_ = consts.tile([P, 6], F32, tag=f"bnc{ob}", name=f"bnc{ob}")
            nc.sync.dma_start(out=c_[:, :], in_=bnq_t[ob * P:(ob + 1) * P, :])
            bnc.append(c_)
        hmask = consts.tile([P, 2], F32, tag="hmask")
        nc.sync.dma_start(out=hmask[:, :], in_=hm_t[:, :])
        hpm = consts.tile([P, 144], F16, tag="hpm")
        nc.gpsimd.dma_start(out=hpm[:, :], in_=hpm_t[:, :])
        offs = consts.tile([1, 18], I32, tag="offs")
        nc.sync.dma_start(out=offs[:, :], in_=offt_t[:, :])
        eps_c = consts.tile([P, 1], F32, tag="eps_c")
        nc.vector.memset(eps_c[:, :], EPS)
        expb_c = consts.tile([P, 1], F32, tag="expb_c")
        nc.vector.memset(expb_c[:, :], EXPB)

        # ---- persistent attention-phase buffers ----
        tri = [persist.tile([P, NB, B, TRW], F16, tag=f"tri{m}",
                            name=f"tri{m}") for m in range(2)]
        x_att = persist.tile([P, NB, B, XAW], F16, tag="x_att")
        hp = [persist.tile([P, NB, B, 9, 16], F16, tag=f"hp{m}",
                           name=f"hp{m}") for m in range(2)]
        acol = persist.tile([P, 4], F32, tag="acol")
        bcol = persist.tile([P, 4], F32, tag="bcol")

        statd = dram.tile([P, 8], F32, tag="statd")
        statg = dram.tile([NCORES * P, 8], F32, tag="statg")
        statd2 = dram.tile([P, 4], F32, tag="statd2")
        statg2 = dram.tile([NCORES * P, 4], F32, tag="statg2")

        def combine_stats(sg, nslots, gmean, gvar):
            """sg [P, nslots(mean,var interleaved j), 8 cores] -> global."""
            msq = tiny.tile([P, nslots, 8], F32, tag="msq")
            mv = sg[:, :, :]  # [P, 2*nslots, 8]
            mean_v = bass.AP(tensor=sg.tensor, offset=sg.offset,
                             ap=[[2 * nslots * 8, P], [16, nslots], [1, 8]])
            var_v = bass.AP(tensor=sg.tensor, offset=sg.offset + 8,
                            ap=[[2 * nslots * 8, P], [16, nslots], [1, 8]])
            nc.vector.tensor_mul(msq[:, :, :], mean_v, mean_v)
            nc.vector.tensor_add(msq[:, :, :], msq[:, :, :], var_v)
            nc.vector.reduce_sum(out=gmean[:, :], in_=mean_v, axis=AX.X)
            nc.vector.reduce_sum(out=gvar[:, :], in_=msq[:, :, :], axis=AX.X)
            nc.vector.tensor_scalar_mul(gmean[:, :], gmean[:, :], 1.0 / 8)
            nc.vector.tensor_scalar_mul(gvar[:, :], gvar[:, :], 1.0 / 8)
            gm2 = tiny.tile([P, nslots], F32, tag="gm2")
            nc.vector.tensor_mul(gm2[:, :], gmean[:, :], gmean[:, :])
            nc.vector.tensor_sub(gvar[:, :], gvar[:, :], gm2[:, :])

        def bn_coeffs(gmean, gvar, acol_, bcol_, gcols, becols):
            """acol = g/sqrt(var+eps); bcol = be - mean*acol (all [P, n])."""
            n = gvar.shape[1]
            sd = tiny.tile([P, n], F32, tag="sd")
            nc.scalar.activation(out=sd[:, :], in_=gvar[:, :], func=AF.Sqrt,
                                 bias=eps_c[:, :])
            nc.vector.reciprocal(sd[:, :], sd[:, :])
            nc.vector.tensor_mul(acol_[:, :], gcols, sd[:, :])
            tmp = tiny.tile([P, n], F32, tag="tmpc")
            nc.vector.tensor_mul(tmp[:, :], gmean[:, :], acol_[:, :])
            nc.vector.tensor_sub(bcol_[:, :], becols, tmp[:, :])

        # ================= phase 1: convs + BN stats ==================
        with ExitStack() as s1:
            rhsp = s1.enter_context(tc.tile_pool(name="rhs", bufs=4))
            psump = s1.enter_context(
                tc.tile_pool(name="psum", bufs=2, space="PSUM"))
            ybuf = s1.enter_context(tc.tile_pool(name="ybuf", bufs=1))
            y = {}
            for m in range(2):
                for ob in range(NB):
                    y[(m, ob)] = ybuf.tile([P, CC], F16, tag=f"y{m}{ob}",
                                           name=f"y{m}{ob}")
            hpraw = [ybuf.tile([P, NB, B, 9, 16], F16, tag=f"hpr{m}",
                               name=f"hpr{m}") for m in range(2)]

            xc = [rhsp.tile([P, CC], F32R, tag=f"xc{ib}", name=f"xc{ib}")
                  for ib in range(NB)]
            for ib in range(NB):
                for h in range(3):
                    nc.sync.dma_start(
                        out=xc[ib][:, h * 1024:(h + 1) * 1024],
                        in_=xs_t[ib * P:(ib + 1) * P,
                                 h * 1024:(h + 1) * 1024])

            NCH = 6
            CW = CC // NCH  # 512
            for chk in range(NCH):
                sl = slice(chk * CW, (chk + 1) * CW)
                for m, wname in ((0, "w1"), (1, "w2")):
                    for ob in range(NB):
                        ps = psump.tile([P, CW], F32, tag="ps", name="ps")
                        for ib in range(NB):
                            nc.tensor.matmul(
                                ps[:, :], wt((wname, ib, ob)),
                                xc[ib][:, sl], start=(ib == 0), stop=(ib == 1),
                                tile_position=(0, 0))
                        nc.scalar.activation(
                            out=y[(m, ob)][:, sl],
                            in_=ps[:, :], func=AF.Copy)

            # x_att: cast slab rows 1..10 into padded layout (early: only
            # needs xc), plus all pad-position zeroing for x_att and tri
            for ib in range(NB):
                nc.vector.tensor_copy(
                    out=bass.AP(
                        tensor=x_att.tensor,
                        offset=x_att.offset + ib * B * XAW + 1,
                        ap=[[NB * B * XAW, P], [XAW, B], [65, RX], [1, HH]]),
                    in_=bass.AP(tensor=xc[ib].tensor,
                                offset=xc[ib].offset + HH,
                                ap=[[CC, P], [768, B], [64, RX],
                                    [1, HH]]).bitcast(F32))
            nc.vector.memset(
                bass.AP(tensor=x_att.tensor, offset=x_att.offset,
                        ap=[[NB * B * XAW, P], [B * XAW, NB], [XAW, B],
                            [65, RX + 1]]), 0.0)
            for m in range(2):
                nc.vector.memset(
                    bass.AP(tensor=tri[m].tensor, offset=tri[m].offset,
                            ap=[[NB * B * TRW, P], [B * TRW, NB], [TRW, B],
                                [65, RH + 1]]), 0.0)

            # patch strips: same convs on xp
            rp = []
            for ib in range(NB):
                r = rhsp.tile([P, NPC], F32R, tag="rhsp", name=f"rhsp{ib}")
                nc.gpsimd.dma_start(out=r[:, :],
                                    in_=xp_t[ib * P:(ib + 1) * P, :])
                rp.append(r)
            for m, wname in ((0, "w1"), (1, "w2")):
                for ob in range(NB):
                    for half in range(2):
                        sl = slice(half * 288, (half + 1) * 288)
                        ps = psump.tile([P, 288], F32, tag="psp", name="psp")
                        for ib in range(NB):
                            nc.tensor.matmul(
                                ps[:, :], wt((wname, ib, ob)),
                                rp[ib][:, sl], start=(ib == 0), stop=(ib == 1),
                                tile_position=(0, 0))
                        nc.scalar.activation(
                            out=hpraw[m][:, ob, 2 * half:2 * half + 2]
                            .rearrange("p b t j -> p (b t j)"),
                            in_=ps[:, :], func=AF.Copy)

            # ---- local BN stats ----
            stt = persist.tile([P, 4, 4, 6], F32, tag="stt")
            msv = persist.tile([P, 4, 2], F32, tag="msv")
            for m in range(2):
                for ob in range(NB):
                    slot = 2 * m + ob
                    for b in range(B):
                        nc.vector.bn_stats(
                            out=stt[:, slot, b, :],
                            in_=y[(m, ob)][:, b * 768 + 128:b * 768 + 640])
                    nc.vector.bn_aggr(out=msv[:, slot, :],
                                      in_=stt[:, slot, :, :])
            nc.sync.dma_start(out=statd[:, :],
                              in_=msv[:, :, :].rearrange("p s v -> p (s v)"))
            nc.gpsimd.collective_compute(
                "AllGather", mybir.AluOpType.bypass,
                replica_groups=[list(range(NCORES))],
                ins=[statd[:, :].opt()], outs=[statg[:, :].opt()])
            sg = persist.tile([P, 8, 8], F32, tag="sg")
            nc.sync.dma_start(
                out=sg[:, :, :],
                in_=bass.AP(tensor=statg.tensor, offset=statg.offset,
                            ap=[[8, P], [1, 8], [8 * P, 8]]))
            gmean = persist.tile([P, 4], F32, tag="gmean")
            gvar = persist.tile([P, 4], F32, tag="gvar")
            combine_stats(sg, 4, gmean, gvar)
            # g/be columns per slot: slot=2m+ob -> bnc[ob][:, 2m], [:, 2m+1]
            gq = persist.tile([P, 4], F32, tag="gq")
            bq = persist.tile([P, 4], F32, tag="bq")
            for m in range(2):
                for ob in range(NB):
                    slot = 2 * m + ob
                    nc.vector.tensor_copy(out=gq[:, slot:slot + 1],
                                          in_=bnc[ob][:, 2 * m:2 * m + 1])
                    nc.vector.tensor_copy(out=bq[:, slot:slot + 1],
                                          in_=bnc[ob][:, 2 * m + 1:2 * m + 2])
            bn_coeffs(gmean, gvar, acol, bcol, gq[:, :], bq[:, :])

            # ---- BN apply + relu -> tri (padded), hp ----
            # Halo rows at the global image edge must be 0 (reference pads
            # tri with zeros).  relu(hm*(a*y+b)) == hm*relu(a*y+b) for
            # hm in {0,1}, so masked coefficients fold the mask into the
            # apply: 3 row bands (top halo, interior, bottom halo).
            acolH = persist.tile([P, 4], F32, tag="acolH")
            bcolH = persist.tile([P, 4], F32, tag="bcolH")
            acolB = persist.tile([P, 4], F32, tag="acolB")
            bcolB = persist.tile([P, 4], F32, tag="bcolB")
            nc.vector.tensor_scalar_mul(acolH[:, :], acol[:, :], hmask[:, 0:1])
            nc.vector.tensor_scalar_mul(bcolH[:, :], bcol[:, :], hmask[:, 0:1])
            nc.vector.tensor_scalar_mul(acolB[:, :], acol[:, :], hmask[:, 1:2])
            nc.vector.tensor_scalar_mul(bcolB[:, :], bcol[:, :], hmask[:, 1:2])
            for ob in range(NB):
                for m in range(2):
                    slot = 2 * m + ob
                    # BN apply on DVE (tensor_scalar affine at fp16 4x,
                    # then relu) so Act is free for the first overcopies
                    for (r0_, nr_, ac_, bc_) in (
                            (2, RH - 4, acol, bcol),
                            (0, 2, acolH, bcolH),
                            (RH - 2, 2, acolB, bcolB)):
                        dst = bass.AP(
                            tensor=tri[m].tensor,
                            offset=(tri[m].offset + ob * B * TRW + 1
                                    + r0_ * 65),
                            ap=[[NB * B * TRW, P], [TRW, B], [65, nr_],
                                [1, HH]])
                        nc.vector.tensor_scalar(
                            out=dst,
                            in0=bass.AP(
                                tensor=y[(m, ob)].tensor,
                                offset=y[(m, ob)].offset + r0_ * HH,
                                ap=[[CC, P], [768, B], [64, nr_], [1, HH]]),
                            scalar1=ac_[:, slot:slot + 1],
                            scalar2=bc_[:, slot:slot + 1],
                            op0=mybir.AluOpType.mult,
                            op1=mybir.AluOpType.add)
                        nc.vector.tensor_scalar_max(dst, dst, 0.0)
                    nc.scalar.activation(
                        out=hp[m][:, ob], in_=hpraw[m][:, ob], func=AF.Relu,
                        bias=bcol[:, slot:slot + 1],
                        scale=acol[:, slot:slot + 1])
                    # zero the patch slots whose source pixel is outside
                    # the image (reference zero-padding)
                    nc.vector.tensor_mul(
                        hp[m][:, ob],
                        hp[m][:, ob],
                        bass.AP(tensor=hpm.tensor, offset=hpm.offset,
                                ap=[[144, P], [0, B], [16, 9], [1, 16]]))
        # ================= phase 2: attention ==================
        # psF accumulates the FINAL conv directly: yf = wf . sum_t Pt
        # (linear), so each iteration's Pt feeds wf-matmuls straight into
        # the final-conv PSUM — no separate pre accumulation/eviction.
        psumA = top.enter_context(
            tc.tile_pool(name="psumA", bufs=1, space="PSUM"))
        psF = [psumA.tile([P, B * LC], F32, tag=f"psF{oo}",
                          name=f"psF{oo}") for oo in range(NB)]
        with ExitStack() as s2:
            fpool = s2.enter_context(tc.tile_pool(name="fpool", bufs=3))
            tpool = s2.enter_context(tc.tile_pool(name="tpool", bufs=3))
            npool = s2.enter_context(tc.tile_pool(name="npool", bufs=3))

            for rho in range(9):
                s_, ng = _geom(rho)
                w9 = 9 * ng
                # per-core shift offsets (registers)
                shoff_a = nc.scalar.value_load(offs[0:1, rho:rho + 1])
                shoff_v = nc.vector.value_load(offs[0:1, rho:rho + 1])
                hoff_a = nc.scalar.value_load(offs[0:1, 9 + rho:10 + rho])
                for ob in range(NB):
                    Fc = []
                    for m in range(2):
                        F_ = fpool.tile([P, B, 640], F16, tag=f"F{m}",
                                        name=f"F{m}c")
                        # overcopy: 10 shifted rows per b (Act)
                        src = bass.AP(
                            tensor=tri[m].tensor,
                            offset=tri[m].offset + ob * B * TRW + shoff_a,
                            ap=[[NB * B * TRW, P], [TRW, B], [65, 10],
                                [1, 64]],
                            dep_tracking_offset=(tri[m].offset
                                                 + ob * B * TRW))
                        dst = F_[:, :, :].rearrange(
                            "p b (r c) -> p b r c", c=64)
                        nc.scalar.activation(out=dst, in_=src, func=AF.Copy)
                        # seam patch (Act): head [56:64) + tail [576:584)
                        nc.scalar.activation(
                            out=bass.AP(tensor=F_.tensor,
                                        offset=F_.offset + 56,
                                        ap=[[B * 640, P], [640, B], [520, 2],
                                            [1, 8]]),
                            in_=bass.AP(
                                tensor=hp[m].tensor,
                                offset=hp[m].offset + ob * B * 144 + hoff_a,
                                ap=[[NB * B * 144, P], [144, B], [8, 2],
                                    [1, 8]],
                                dep_tracking_offset=(hp[m].offset
                                                     + ob * B * 144)),
                            func=AF.Copy)
                        Fc.append(F_)
                    t1 = tpool.tile([P, B, 528], F16, tag="t1", name="t1")
                    t2 = tpool.tile([P, B, 528], F16, tag="t2", name="t2")
                    lgb = tpool.tile([P, B, 528], F16, tag="lgb", name="lgb")
                    Ne = npool.tile([P, B, 528], F32, tag="Ne", name="Ne")
                    Z = npool.tile([P, B, 64], F32, tag="Z", name="Z")
                    R = npool.tile([P, B, 64], F32, tag="R", name="R")
                    NN = npool.tile([P, B, 528], F16, tag="NN", name="NN")
                    Pt = npool.tile([P, B, LC], F16, tag="Pt", name="Pt")

                    def gv(tl, off, b0, nb):
                        return bass.AP(
                            tensor=tl.tensor,
                            offset=tl.offset + tl.shape[2] * b0 + off,
                            ap=[[tl.shape[1] * tl.shape[2], P],
                                [tl.shape[2], nb], [9, ng], [1, 9]])

                    def cbv(tl, off, b0, nb):
                        return bass.AP(
                            tensor=tl.tensor,
                            offset=tl.offset + tl.shape[2] * b0 + off,
                            ap=[[tl.shape[1] * tl.shape[2], P],
                                [tl.shape[2], nb], [9, ng], [0, 9]])

                    # last phase runs per-b to shorten the drain chain
                    bsl = [(b, 1) for b in range(B)] if rho == 8 else [(0, B)]
                    for (b0, nb) in bsl:
                        # center-broadcast muls: t1 on Pool; t2 split
                        # Pool (b<3) / DVE (b=3) to balance the pacer
                        nc.gpsimd.tensor_mul(
                            gv(t1, 0, b0, nb), gv(Fc[0], 64 - rho, b0, nb),
                            cbv(Fc[1], 64 - rho + 4, b0, nb))
                        for (c0, nb2, eng) in (((b0, min(nb, 3), nc.gpsimd),
                                                (3, 1, nc.vector))
                                               if (b0 == 0 and nb == B) else
                                               ((b0, nb,
                                                 nc.vector if b0 == 3
                                                 else nc.gpsimd),)):
                            eng.tensor_mul(
                                gv(t2, 0, c0, nb2),
                                gv(Fc[1], 64 - rho, c0, nb2),
                                cbv(Fc[0], 64 - rho + 4, c0, nb2))
                        nc.vector.tensor_add(
                            lgb[:, b0:b0 + nb, :w9],
                            t1[:, b0:b0 + nb, :w9], t2[:, b0:b0 + nb, :w9])
                        nc.scalar.activation(out=Ne[:, b0:b0 + nb, :w9],
                                             in_=lgb[:, b0:b0 + nb, :w9],
                                             func=AF.Exp, bias=expb_c[:, :])
                        nc.vector.reduce_sum(
                            out=Z[:, b0:b0 + nb, :ng],
                            in_=Ne[:, b0:b0 + nb, :w9].rearrange(
                                "p b (g s) -> p b g s", s=9),
                            axis=AX.X)
                        nc.vector.reciprocal(R[:, b0:b0 + nb, :ng],
                                             Z[:, b0:b0 + nb, :ng])
                        nc.vector.tensor_mul(
                            NN[:, b0:b0 + nb, :w9].rearrange(
                                "p b (g s) -> p b g s", s=9),
                            Ne[:, b0:b0 + nb, :w9].rearrange(
                                "p b (g s) -> p b g s", s=9),
                            bass.AP(tensor=R.tensor,
                                    offset=R.offset + 64 * b0,
                                    ap=[[B * 64, P], [64, nb], [1, ng],
                                        [0, 9]]))
                        nc.vector.tensor_mul(
                            Pt[:, b0:b0 + nb, :],
                            NN[:, b0:b0 + nb, rho:rho + LC],
                            bass.AP(tensor=x_att.tensor,
                                    offset=(x_att.offset + ob * B * XAW
                                            + XAW * b0 + shoff_v),
                                    ap=[[NB * B * XAW, P], [XAW, nb],
                                        [65, RO], [1, HH]],
                                    dep_tracking_offset=(x_att.offset
                                                         + ob * B * XAW)))
                        for q in range(b0, b0 + nb):
                            for oo in range(NB):
                                nc.tensor.matmul(
                                    psF[oo][:, q * LC:(q + 1) * LC],
                                    wf16[(ob, oo)][:, :],
                                    Pt[:, q, :],
                                    start=(rho == 0 and ob == 0),
                                    stop=(rho == 8 and ob == NB - 1),
                                    tile_position=(0, 0),
                                    skip_group_check=True)

        # ================= phase 3: final conv + BNf ==================
        with ExitStack() as s3:
            fbuf = s3.enter_context(tc.tile_pool(name="fbuf", bufs=1))
            yf = [fbuf.tile([P, B * LC], F16, tag=f"yf{ob}",
                            name=f"yf{ob}") for ob in range(NB)]
            stf = persist.tile([P, 2, 4, 6], F32, tag="stf")
            msvf = persist.tile([P, 2, 2], F32, tag="msvf")
            # quarter-pipelined evict + stats (stats read PSUM directly,
            # in parallel with the Act evict)
            for q in range(B):
                sl = slice(q * LC, (q + 1) * LC)
                for ob in range(NB):
                    nc.scalar.activation(out=yf[ob][:, sl], in_=psF[ob][:, sl],
                                         func=AF.Copy)
                    nc.vector.bn_stats(out=stf[:, ob, q, :],
                                       in_=psF[ob][:, sl])
            for ob in range(NB):
                nc.vector.bn_aggr(out=msvf[:, ob, :], in_=stf[:, ob, :, :])
            nc.sync.dma_start(out=statd2[:, :],
                              in_=msvf[:, :, :].rearrange("p s v -> p (s v)"))
            nc.gpsimd.collective_compute(
                "AllGather", mybir.AluOpType.bypass,
                replica_groups=[list(range(NCORES))],
                ins=[statd2[:, :].opt()], outs=[statg2[:, :].opt()])
            sg2 = persist.tile([P, 4, 8], F32, tag="sg2")
            nc.sync.dma_start(
                out=sg2[:, :, :],
                in_=bass.AP(tensor=statg2.tensor, offset=statg2.offset,
                            ap=[[4, P], [1, 4], [4 * P, 8]]))
            gmean2 = persist.tile([P, 2], F32, tag="gmean2")
            gvar2 = persist.tile([P, 2], F32, tag="gvar2")
            combine_stats(sg2, 2, gmean2, gvar2)
            gq2 = persist.tile([P, 2], F32, tag="gq2")
            bq2 = persist.tile([P, 2], F32, tag="bq2")
            for ob in range(NB):
                nc.vector.tensor_copy(out=gq2[:, ob:ob + 1],
                                      in_=bnc[ob][:, 4:5])
                nc.vector.tensor_copy(out=bq2[:, ob:ob + 1],
                                      in_=bnc[ob][:, 5:6])
            acolf = persist.tile([P, 2], F32, tag="acolf")
            bcolf = persist.tile([P, 2], F32, tag="bcolf")
            bn_coeffs(gmean2, gvar2, acolf, bcolf, gq2[:, :], bq2[:, :])
            fout = [fbuf.tile([P, B * LC], F32, tag=f"fout{ob}",
                              name=f"fout{ob}") for ob in range(NB)]
            for b in range(B):
                sl = slice(b * LC, (b + 1) * LC)
                for ob in range(NB):
                    nc.scalar.activation(out=fout[ob][:, sl],
                                         in_=yf[ob][:, sl],
                                         func=AF.Relu,
                                         bias=bcolf[:, ob:ob + 1],
                                         scale=acolf[:, ob:ob + 1])
                    nc.sync.dma_start(out=out_t[ob * P:(ob + 1) * P, sl],
                                      in_=fout[ob][:, sl])

    nc.finalize()
    return nc


_NC_CACHE = None


def _get_nc():
    global _NC_CACHE
    if _NC_CACHE is None:
        _NC_CACHE = build_program()
    return _NC_CACHE


def make_in_maps(inputs):
    x = np.asarray(inputs["x"], np.float32).reshape(B, C, HH, HH)
    w1t = np.asarray(inputs["w1"], np.float32).T
    w2t = np.asarray(inputs["w2"], np.float32).T
    wft = np.asarray(inputs["wf"], np.float32).T
    wts = np.ascontiguousarray(
        np.concatenate([w1t, w2t, wft], axis=1))  # [256, 768]
    bnq = np.ascontiguousarray(np.stack([
        np.asarray(inputs["g1"], np.float32),
        np.asarray(inputs["be1"], np.float32),
        np.asarray(inputs["g2"], np.float32),
        np.asarray(inputs["be2"], np.float32),
        np.asarray(inputs["gf"], np.float32),
        np.asarray(inputs["bef"], np.float32),
    ], axis=1))  # [256, 6]

    shifts = [(_shift(t)) for t in range(9)]
    maps = []
    for k in range(NCORES):
        r0 = RO * k
        # xs slab: rows r0-2 .. r0+9, zero outside image
        xs = np.zeros((NB * P, CC), np.float32)
        for rr in range(RH):
            gr = r0 - 2 + rr
            if 0 <= gr < HH:
                # xs[ib*128+ci, b*768 + rr*64 + cc]
                blk = x[:, :, gr, :]  # [B, C, 64]
                for ib in range(NB):
                    xs[ib * P:(ib + 1) * P,
                       np.arange(B)[:, None] * 768 + rr * 64
                       + np.arange(HH)[None, :]] = \
                        blk[:, ib * P:(ib + 1) * P, :].transpose(1, 0, 2)
        # patch strips xp [2P, B*9*16] + validity mask hpmv [P, 144]
        xp = np.zeros((NB * P, NPC), np.float32)
        hpmv = np.zeros((P, 144), np.float16)
        for t in range(9):
            for jj in range(16):
                if jj < 8:
                    l = LC * k - 8 + jj
                    ts, lp = (t, l) if l >= 0 else (t - 1, l + L)
                else:
                    l = LC * k + LC + (jj - 8)
                    ts, lp = (t, l) if l < L else (t + 1, l - L)
                if ts < 0 or ts > 8:
                    continue
                di, dj = shifts[ts]
                rr_, cc_ = lp // HH + di, lp % HH + dj
                if 0 <= rr_ < HH and 0 <= cc_ < HH:
                    hpmv[:, t * 16 + jj] = 1.0
                    col = np.arange(B) * 144 + t * 16 + jj
                    val = x[:, :, rr_, cc_]  # [B, C]
                    for ib in range(NB):
                        xp[ib * P:(ib + 1) * P, col] = \
                            val[:, ib * P:(ib + 1) * P].T
        hmask = np.ones((P, 2), np.float32)
        if k == 0:
            hmask[:, 0] = 0.0
        if k == NCORES - 1:
            hmask[:, 1] = 0.0
        offt = np.zeros((1, 18), np.int32)
        for rho in range(9):
            t = (rho + k) % 9
            di, dj = shifts[t]
            offt[0, rho] = 66 + 65 * di + dj
            offt[0, 9 + rho] = 16 * t
        maps.append({
            "xs": xs, "xp": xp, "wts": wts,
            "bnq": bnq, "hmask": hmask, "offt": offt, "hpmv": hpmv,
        })
    return maps


def run(inputs, trace=False):
    nc = _get_nc()
    in_maps = make_in_maps(inputs)
    res = run_bass_kernel_spmd(nc, in_maps, core_ids=list(range(NCORES)),
                               trace=trace)
    full = np.empty((B, C, HH, HH), np.float32)
    for k in range(NCORES):
        o = res.results[k]["out"].reshape(NB, P, B, RO, HH)
        for ob in range(NB):
            full[:, ob * P:(ob + 1) * P, RO * k:RO * (k + 1), :] = \
                o[ob].transpose(1, 0, 2, 3)
    return full, res


def kernel(**inputs) -> np.ndarray:
    out, _ = run(inputs, trace=False)
    return out
